# revision 18
# baseline (speedup 1.0000x reference)
"""Circulant-matmul kernel for Trainium2 (8 NeuronCores, SPMD).

Problem: out[b, i, d] = sum_m alpha[(i - m) mod N] * x[b, m, d]
with x: [2, 8192, 32] fp32, alpha: [8192] fp32.

Strategy (v10, raw bacc, bf16)
------------------------------
Same math as v9: flatten x to X[m, f] (f = b*32 + d, F = 64), shard output
tokens across 8 cores (core c rows [1024c, 1024c+1024)), host-rotate alpha
so every core runs the identical program. 64 accumulating full-array bf16
matmuls compute psum[(h, f), q] = out.T with the pair-stationary trick
([X_j | X_{j+4}] against a 512-wide skewed-alpha moving slice).

v10 changes, driven by the v9 trace (exec 33-38 us):
 * NO TileContext. The Tile scheduler's semaphore plumbing added ~300
   EVENT_SEMAPHORE instructions, ~9 us of which executed AFTER the last
   output DMA and inside the graded exec window (exec_time = last inst end
   minus first engine-op start). Raw bacc with 8 hand-placed semaphores has
   a ~0.5 us tail instead.
 * Paired stationaries [X_j | X_{j+4}] are built by 6 DVE spread-copies
   (BIR requires the matmul stationary AP to have ONE free dim, so the
   direct 3-dim-AP read of xsrc is not allowed), gated per x-chunk with
   one shared semaphore so the PE waits at exactly 3 pair boundaries.
 * wbd is indexed so MM k's moving slice is wbuf[:, 128k : 128k+512] for
   ALL k (the k=63 wrap is materialized at the top of the host view):
   wbd[p, j] = ac[(j + p + 1) mod N], 8576 cols. 3 contiguous wbuf chunks
   + 2 xsrc chunks = 5 input dma_starts (each costs ~0.65 us of serial
   HWDGE descriptor generation, so fewer and bigger is better), issued
   A1,B1,A2,B2,A3 in consumption order across both HWDGE rings.
 * PE waits on the chunk semaphores only at the 4 consumption boundaries
   (k = 0, 13, 24, 37); everything else rides program order.
 * 8 warmup matmuls on a zeroed tile bridge the ~3.4 us HAM un-throttle
   window while the first chunks stream, so real MMs run at 2.4 GHz
   (216 ns per 512-wide bf16 matmul) with no mid-stream re-throttle.
 * psum is drained by two DVE copies (bf16 downcast) feeding one output
   DMA per ring; the program ends on two sync waits for the output DMA
   completion sems.
"""

import os
import sys

import numpy as np

for _p in ("/opt/trn_rl_repo",):
    if os.path.isdir(_p) and _p not in sys.path:
        sys.path.insert(0, _p)

from ml_dtypes import bfloat16

import concourse.bass as bass
from concourse import bacc, bass_utils, mybir

N = 8192          # token axis
P = 128           # SBUF partitions / matmul contraction tile
F = 64            # packed feature dim (B * D = 2 * 32)
NM = N // P       # 64 m-blocks
JW = 8576         # wbuf columns: MM k reads [128k, 128k+512), k = 0..63
NCORES = 8
NI = N // NCORES  # 1024 output rows per core
DT = mybir.dt.bfloat16
WARMUP_MM = 8
XSRC_W = (NM + 4) * F  # 4352 (64 blocks + 4 wrap-pad blocks)

# wbuf chunk ends (cols): A1 covers MM k<=12, A2 k<=36, A3 k<=63.
WB_CUTS = (2048, 5120, JW)
# xsrc chunk block boundaries (B1 = [40,68), B2 = [16,40), B3 = [0,16))
# and the pair groups each spread produces.
XCHUNKS = ((40, 68), (16, 40), (0, 16))
PAIRS = ((40, 64), (16, 40), (0, 16))

_cache = {}


def _build():
    nc = bacc.Bacc(
        "TRN2", target_bir_lowering=False, debug=False, num_devices=NCORES
    )
    xin = nc.dram_tensor("xin", [P, XSRC_W], DT, kind="ExternalInput")
    wbd = nc.dram_tensor("wbd", [P, JW], DT, kind="ExternalInput")
    yout = nc.dram_tensor("yout", [P, 512], DT, kind="ExternalOutput")

    wbuf = nc.alloc_sbuf_tensor("wbuf", [P, JW], DT)
    xsrc = nc.alloc_sbuf_tensor("xsrc", [P, XSRC_W], DT)
    xpair = nc.alloc_sbuf_tensor("xpair", [P, P * NM], DT)
    obuf = nc.alloc_sbuf_tensor("obuf", [P, 512], DT)
    wrm = nc.alloc_sbuf_tensor("wrm", [P, 512], DT)
    ps = nc.alloc_psum_tensor("ps", [P, 512], mybir.dt.float32)
    ps_wrm = nc.alloc_psum_tensor("ps_wrm", [P, 512], mybir.dt.float32)

    s_m = nc.alloc_semaphore("s_m")    # DVE memset -> PE warmup
    s_a = nc.alloc_semaphore("s_a")    # ring A (sync) input chunks
    s_b = nc.alloc_semaphore("s_b")    # ring B (scalar) input chunks
    s_x = nc.alloc_semaphore("s_x")    # spread group done -> PE
    s_pe = nc.alloc_semaphore("s_pe")  # last MM -> DVE drain
    s_c0 = nc.alloc_semaphore("s_c0")  # cast half 0 -> out DMA A
    s_c1 = nc.alloc_semaphore("s_c1")  # cast half 1 -> out DMA B
    s_oa = nc.alloc_semaphore("s_oa")  # out DMA A done
    s_ob = nc.alloc_semaphore("s_ob")  # out DMA B done

    # DVE: zero the warmup stationary/moving tile first thing.
    nc.vector.memset(wrm[:, :], 0.0).then_inc(s_m)

    # Input DMAs, interleaved across the two HWDGE rings in consumption
    # order (each dma_start serializes ~0.65us of descriptor generation on
    # the shared HWDGE block). B1 first: its spread gates the first MM.
    c0, c1, c2 = WB_CUTS
    (b1l, b1h), (b2l, b2h), (b3l, b3h) = XCHUNKS
    nc.scalar.dma_start(
        out=xsrc[:, F * b1l : F * b1h], in_=xin.ap()[:, F * b1l : F * b1h]
    ).then_inc(s_b, 16)
    nc.sync.dma_start(out=wbuf[:, 0:c0], in_=wbd.ap()[:, 0:c0]).then_inc(s_a, 16)
    nc.sync.dma_start(out=wbuf[:, c0:c1], in_=wbd.ap()[:, c0:c1]).then_inc(s_a, 16)
    nc.scalar.dma_start(
        out=xsrc[:, F * b2l : F * b2h], in_=xin.ap()[:, F * b2l : F * b2h]
    ).then_inc(s_b, 16)
    nc.scalar.dma_start(
        out=xsrc[:, F * b3l : F * b3h], in_=xin.ap()[:, F * b3l : F * b3h]
    ).then_inc(s_b, 16)
    nc.sync.dma_start(out=wbuf[:, c1:c2], in_=wbd.ap()[:, c1:c2]).then_inc(s_a, 16)

    # DVE spread-copies build the paired stationaries from each x chunk:
    #   xpair[:, 128j + u]      = xsrc[:, 64j + u]          (u < 64)
    #   xpair[:, 128j + 64 + u] = xsrc[:, 64(j+4) + u]
    xp = xpair[:, :]
    xs = xsrc[:, :]
    for gi, (plo, phi) in enumerate(PAIRS):
        nc.vector.wait_ge(s_b, 16 * (gi + 1))
        nblk = phi - plo
        last = None
        for half, off in ((0, 0), (1, 4 * F)):
            last = nc.vector.tensor_copy(
                bass.AP(
                    xp.tensor,
                    xp.offset + 2 * F * plo + F * half,
                    [[P * NM, P], [2 * F, nblk], [1, F]],
                ),
                bass.AP(
                    xs.tensor,
                    xs.offset + F * plo + off,
                    [[XSRC_W, P], [F, nblk], [1, F]],
                ),
            )
        last.then_inc(s_x)

    # PE warmup: keep the array busy so HAM un-throttles (~3.4us) right as
    # the first input chunks land.
    nc.tensor.wait_ge(s_m, 1)
    for w in range(WARMUP_MM):
        nc.tensor.matmul(
            ps_wrm[:, :],
            lhsT=wrm[:, 0:128],
            rhs=wrm[:, :],
            start=(w == 0),
            stop=(w == WARMUP_MM - 1),
        )

    # Real matmuls: 64 accumulating steps, stationary xpair[:, 128j:128j+128].
    nc.tensor.wait_ge(s_x, 1)
    nc.tensor.wait_ge(s_a, 16)
    mm = None
    for k in range(NM):
        j = NM - 1 - k
        if k == 13:
            nc.tensor.wait_ge(s_a, 32)
        if k == NM - PAIRS[0][0]:   # k=24: pairs j<40 from spread group 2
            nc.tensor.wait_ge(s_x, 2)
        if k == 37:
            nc.tensor.wait_ge(s_a, 48)
        if k == NM - PAIRS[1][0]:   # k=48: pairs j<16 from spread group 3
            nc.tensor.wait_ge(s_x, 3)
        mm = nc.tensor.matmul(
            ps[:, :],
            lhsT=xpair[:, P * j : P * (j + 1)],
            rhs=wbuf[:, P * k : P * k + 512],
            start=(k == 0),
            stop=(k == NM - 1),
        )
    mm.then_inc(s_pe)

    # Drain psum (fp32 -> bf16) on DVE, one half per output DMA ring.
    nc.vector.wait_ge(s_pe, 1)
    nc.vector.tensor_copy(obuf[:, 0:256], ps[:, 0:256]).then_inc(s_c0)
    nc.vector.tensor_copy(obuf[:, 256:512], ps[:, 256:512]).then_inc(s_c1)

    nc.sync.wait_ge(s_c0, 1)
    nc.sync.dma_start(out=yout.ap()[:, 0:256], in_=obuf[:, 0:256]).then_inc(
        s_oa, 16
    )
    nc.scalar.wait_ge(s_c1, 1)
    nc.scalar.dma_start(
        out=yout.ap()[:, 256:512], in_=obuf[:, 256:512]
    ).then_inc(s_ob, 16)

    # Hold the NEFF open until both output DMAs have landed.
    nc.sync.wait_ge(s_oa, 16)
    nc.sync.wait_ge(s_ob, 16)

    nc.compile()
    return nc


def _prep_in_maps(x, alpha_delta):
    X = np.ascontiguousarray(x.transpose(1, 0, 2).reshape(N, F)).astype(bfloat16)
    # Xb[M, p, f] = X[128M + 127 - p, f]   (reversed r-within-block)
    Xb = X.reshape(NM, P, F)[:, ::-1, :]
    Xb = np.concatenate([Xb, Xb[:4]], axis=0)  # wrap pad: X_0..X_3
    xin = np.ascontiguousarray(Xb.transpose(1, 0, 2).reshape(P, XSRC_W))
    in_maps = []
    for c in range(NCORES):
        ac = np.roll(alpha_delta, -NI * c)
        a2 = np.ascontiguousarray(
            np.concatenate([ac, ac, ac[:512]]).astype(bfloat16)
        )
        # host-side skew: wbd[p, j] = a2[N + 1 + p + j], zero-copy view
        wbd = np.ascontiguousarray(
            np.lib.stride_tricks.as_strided(
                a2[N + 1:], shape=(P, JW), strides=(2, 2)
            )
        )
        in_maps.append({"xin": xin, "wbd": wbd})
    return in_maps


def get_nc():
    if "nc" not in _cache:
        _cache["nc"] = _build()
    return _cache["nc"]


def run(x, alpha_delta, **kwargs):
    """Run on hardware; returns (out [2, N, 32], BassKernelResults)."""
    x = np.asarray(x, dtype=np.float32)
    alpha_delta = np.asarray(alpha_delta, dtype=np.float32)
    res = bass_utils.run_bass_kernel_spmd(
        get_nc(), _prep_in_maps(x, alpha_delta), core_ids=list(range(NCORES)),
        **kwargs,
    )
    out = np.empty((N, F), np.float32)
    for c in range(NCORES):
        y = np.asarray(res.results[c]["yout"]).astype(np.float32)  # [128, 512]
        out[c * NI : c * NI + 512, :] = y[:F, :].T
        out[c * NI + 512 : (c + 1) * NI, :] = y[F:, :].T
    out = np.ascontiguousarray(out.reshape(N, 2, 32).transpose(1, 0, 2))
    return out, res


def kernel(x, alpha_delta):
    out, _ = run(x, alpha_delta)
    return out


# revision 19
# speedup vs baseline: 1.0824x; 1.0824x over previous
"""Circulant-matmul kernel for Trainium2 (8 NeuronCores, SPMD).

Problem: out[b, i, d] = sum_m alpha[(i - m) mod N] * x[b, m, d]
with x: [2, 8192, 32] fp32, alpha: [8192] fp32.

Strategy (v10, raw bacc, bf16)
------------------------------
Same math as v9: flatten x to X[m, f] (f = b*32 + d, F = 64), shard output
tokens across 8 cores (core c rows [1024c, 1024c+1024)), host-rotate alpha
so every core runs the identical program. 64 accumulating full-array bf16
matmuls compute psum[(h, f), q] = out.T with the pair-stationary trick
([X_j | X_{j+4}] against a 512-wide skewed-alpha moving slice).

v10 changes, driven by the v9 trace (exec 33-38 us):
 * NO TileContext. The Tile scheduler's semaphore plumbing added ~300
   EVENT_SEMAPHORE instructions, ~9 us of which executed AFTER the last
   output DMA and inside the graded exec window (exec_time = last inst end
   minus first engine-op start). Raw bacc with 8 hand-placed semaphores has
   a ~0.5 us tail instead.
 * Paired stationaries [X_j | X_{j+4}] are built by 6 DVE spread-copies
   (BIR requires the matmul stationary AP to have ONE free dim, so the
   direct 3-dim-AP read of xsrc is not allowed), gated per x-chunk with
   one shared semaphore so the PE waits at exactly 3 pair boundaries.
 * wbd is indexed so MM k's moving slice is wbuf[:, 128k : 128k+512] for
   ALL k (the k=63 wrap is materialized at the top of the host view):
   wbd[p, j] = ac[(j + p + 1) mod N], 8576 cols. 3 contiguous wbuf chunks
   + 2 xsrc chunks = 5 input dma_starts (each costs ~0.65 us of serial
   HWDGE descriptor generation, so fewer and bigger is better), issued
   A1,B1,A2,B2,A3 in consumption order across both HWDGE rings.
 * PE waits on the chunk semaphores only at the 4 consumption boundaries
   (k = 0, 13, 24, 37); everything else rides program order.
 * 8 warmup matmuls on a zeroed tile bridge the ~3.4 us HAM un-throttle
   window while the first chunks stream, so real MMs run at 2.4 GHz
   (216 ns per 512-wide bf16 matmul) with no mid-stream re-throttle.
 * psum is drained by two DVE copies (bf16 downcast) feeding one output
   DMA per ring; the program ends on two sync waits for the output DMA
   completion sems.
"""

import os
import sys

import numpy as np

for _p in ("/opt/trn_rl_repo",):
    if os.path.isdir(_p) and _p not in sys.path:
        sys.path.insert(0, _p)

from ml_dtypes import bfloat16

import concourse.bass as bass
from concourse import bacc, bass_utils, mybir

N = 8192          # token axis
P = 128           # SBUF partitions / matmul contraction tile
F = 64            # packed feature dim (B * D = 2 * 32)
NM = N // P       # 64 m-blocks
JW = 8576         # wbuf columns: MM k reads [128k, 128k+512), k = 0..63
NCORES = 8
NI = N // NCORES  # 1024 output rows per core
DT = mybir.dt.bfloat16
# Warmup must bridge from ~7us (DVE memset done) to the first real MM
# (~13us, gated by the B1+spread chain): ~8 cold MMs @427ns until HAM
# un-throttles, then warm @213ns. 18 keeps the PE busy right up to the
# data arrival so the real stream starts warm (the v10b 2.3us PE-idle
# gap re-throttled HAM and cost ~1.5us of cold real MMs).
WARMUP_MM = 18
XSRC_W = (NM + 4) * F  # 4352 (64 blocks + 4 wrap-pad blocks)

# wbuf chunk ends (cols): A1 covers MM k<=12, A2 k<=36, A3 k<=63.
WB_CUTS = (2048, 5120, JW)
# xsrc chunk block boundaries (B1 = [44,68), B2 = [16,44), B3 = [0,16))
# and the pair groups each spread produces. B1 is the first-MM gate:
# smaller B1 -> earlier T0.
XCHUNKS = ((44, 68), (16, 44), (0, 16))
PAIRS = ((44, 64), (16, 44), (0, 16))

_cache = {}


def _build():
    nc = bacc.Bacc(
        "TRN2", target_bir_lowering=False, debug=False, num_devices=NCORES
    )
    xin = nc.dram_tensor("xin", [P, XSRC_W], DT, kind="ExternalInput")
    wbd = nc.dram_tensor("wbd", [P, JW], DT, kind="ExternalInput")
    yout = nc.dram_tensor("yout", [P, 512], DT, kind="ExternalOutput")

    wbuf = nc.alloc_sbuf_tensor("wbuf", [P, JW], DT)
    xsrc = nc.alloc_sbuf_tensor("xsrc", [P, XSRC_W], DT)
    xpair = nc.alloc_sbuf_tensor("xpair", [P, P * NM], DT)
    obuf = nc.alloc_sbuf_tensor("obuf", [P, 512], DT)
    wrm = nc.alloc_sbuf_tensor("wrm", [P, 512], DT)
    ps = nc.alloc_psum_tensor("ps", [P, 512], mybir.dt.float32)
    ps_wrm = nc.alloc_psum_tensor("ps_wrm", [P, 512], mybir.dt.float32)

    s_m = nc.alloc_semaphore("s_m")    # DVE memset -> PE warmup
    s_a = nc.alloc_semaphore("s_a")    # ring A (sync) input chunks
    s_b = nc.alloc_semaphore("s_b")    # ring B (scalar) input chunks
    s_x = nc.alloc_semaphore("s_x")    # spread group done -> PE
    s_pe = nc.alloc_semaphore("s_pe")  # last MM -> DVE drain
    s_c0 = nc.alloc_semaphore("s_c0")  # cast half 0 -> out DMA A
    s_c1 = nc.alloc_semaphore("s_c1")  # cast half 1 -> out DMA B
    s_oa = nc.alloc_semaphore("s_oa")  # out DMA A done
    s_ob = nc.alloc_semaphore("s_ob")  # out DMA B done

    # DVE: zero the warmup stationary/moving tile first thing.
    nc.vector.memset(wrm[:, :], 0.0).then_inc(s_m)

    # Input DMAs, interleaved across the two HWDGE rings in consumption
    # order (each dma_start serializes ~0.65us of descriptor generation on
    # the shared HWDGE block). B1 first: its spread gates the first MM.
    c0, c1, c2 = WB_CUTS
    (b1l, b1h), (b2l, b2h), (b3l, b3h) = XCHUNKS
    nc.scalar.dma_start(
        out=xsrc[:, F * b1l : F * b1h], in_=xin.ap()[:, F * b1l : F * b1h]
    ).then_inc(s_b, 16)
    nc.sync.dma_start(out=wbuf[:, 0:c0], in_=wbd.ap()[:, 0:c0]).then_inc(s_a, 16)
    nc.sync.dma_start(out=wbuf[:, c0:c1], in_=wbd.ap()[:, c0:c1]).then_inc(s_a, 16)
    nc.scalar.dma_start(
        out=xsrc[:, F * b2l : F * b2h], in_=xin.ap()[:, F * b2l : F * b2h]
    ).then_inc(s_b, 16)
    nc.scalar.dma_start(
        out=xsrc[:, F * b3l : F * b3h], in_=xin.ap()[:, F * b3l : F * b3h]
    ).then_inc(s_b, 16)
    nc.sync.dma_start(out=wbuf[:, c1:c2], in_=wbd.ap()[:, c1:c2]).then_inc(s_a, 16)

    # DVE spread-copies build the paired stationaries from each x chunk:
    #   xpair[:, 128j + u]      = xsrc[:, 64j + u]          (u < 64)
    #   xpair[:, 128j + 64 + u] = xsrc[:, 64(j+4) + u]
    xp = xpair[:, :]
    xs = xsrc[:, :]
    for gi, (plo, phi) in enumerate(PAIRS):
        nc.vector.wait_ge(s_b, 16 * (gi + 1))
        nblk = phi - plo
        last = None
        for half, off in ((0, 0), (1, 4 * F)):
            last = nc.vector.tensor_copy(
                bass.AP(
                    xp.tensor,
                    xp.offset + 2 * F * plo + F * half,
                    [[P * NM, P], [2 * F, nblk], [1, F]],
                ),
                bass.AP(
                    xs.tensor,
                    xs.offset + F * plo + off,
                    [[XSRC_W, P], [F, nblk], [1, F]],
                ),
            )
        last.then_inc(s_x)

    # PE warmup: keep the array busy so HAM un-throttles (~3.4us) right as
    # the first input chunks land.
    nc.tensor.wait_ge(s_m, 1)
    for w in range(WARMUP_MM):
        nc.tensor.matmul(
            ps_wrm[:, :],
            lhsT=wrm[:, 0:128],
            rhs=wrm[:, :],
            start=(w == 0),
            stop=(w == WARMUP_MM - 1),
        )

    # Real matmuls: 64 accumulating steps, stationary xpair[:, 128j:128j+128].
    nc.tensor.wait_ge(s_x, 1)
    nc.tensor.wait_ge(s_a, 16)
    mm = None
    for k in range(NM):
        j = NM - 1 - k
        if k == 13:
            nc.tensor.wait_ge(s_a, 32)
        if k == NM - PAIRS[0][0]:   # k=24: pairs j<40 from spread group 2
            nc.tensor.wait_ge(s_x, 2)
        if k == 37:
            nc.tensor.wait_ge(s_a, 48)
        if k == NM - PAIRS[1][0]:   # k=48: pairs j<16 from spread group 3
            nc.tensor.wait_ge(s_x, 3)
        mm = nc.tensor.matmul(
            ps[:, :],
            lhsT=xpair[:, P * j : P * (j + 1)],
            rhs=wbuf[:, P * k : P * k + 512],
            start=(k == 0),
            stop=(k == NM - 1),
        )
    mm.then_inc(s_pe)

    # Drain psum (fp32 -> bf16) on DVE, one half per output DMA ring.
    nc.vector.wait_ge(s_pe, 1)
    nc.vector.tensor_copy(obuf[:, 0:256], ps[:, 0:256]).then_inc(s_c0)
    nc.vector.tensor_copy(obuf[:, 256:512], ps[:, 256:512]).then_inc(s_c1)

    nc.sync.wait_ge(s_c0, 1)
    nc.sync.dma_start(out=yout.ap()[:, 0:256], in_=obuf[:, 0:256]).then_inc(
        s_oa, 16
    )
    nc.scalar.wait_ge(s_c1, 1)
    nc.scalar.dma_start(
        out=yout.ap()[:, 256:512], in_=obuf[:, 256:512]
    ).then_inc(s_ob, 16)

    # Hold the NEFF open until both output DMAs have landed.
    nc.sync.wait_ge(s_oa, 16)
    nc.sync.wait_ge(s_ob, 16)

    nc.compile()
    return nc


def _prep_in_maps(x, alpha_delta):
    X = np.ascontiguousarray(x.transpose(1, 0, 2).reshape(N, F)).astype(bfloat16)
    # Xb[M, p, f] = X[128M + 127 - p, f]   (reversed r-within-block)
    Xb = X.reshape(NM, P, F)[:, ::-1, :]
    Xb = np.concatenate([Xb, Xb[:4]], axis=0)  # wrap pad: X_0..X_3
    xin = np.ascontiguousarray(Xb.transpose(1, 0, 2).reshape(P, XSRC_W))
    in_maps = []
    for c in range(NCORES):
        ac = np.roll(alpha_delta, -NI * c)
        a2 = np.ascontiguousarray(
            np.concatenate([ac, ac, ac[:512]]).astype(bfloat16)
        )
        # host-side skew: wbd[p, j] = a2[N + 1 + p + j], zero-copy view
        wbd = np.ascontiguousarray(
            np.lib.stride_tricks.as_strided(
                a2[N + 1:], shape=(P, JW), strides=(2, 2)
            )
        )
        in_maps.append({"xin": xin, "wbd": wbd})
    return in_maps


def get_nc():
    if "nc" not in _cache:
        _cache["nc"] = _build()
    return _cache["nc"]


def run(x, alpha_delta, **kwargs):
    """Run on hardware; returns (out [2, N, 32], BassKernelResults)."""
    x = np.asarray(x, dtype=np.float32)
    alpha_delta = np.asarray(alpha_delta, dtype=np.float32)
    res = bass_utils.run_bass_kernel_spmd(
        get_nc(), _prep_in_maps(x, alpha_delta), core_ids=list(range(NCORES)),
        **kwargs,
    )
    out = np.empty((N, F), np.float32)
    for c in range(NCORES):
        y = np.asarray(res.results[c]["yout"]).astype(np.float32)  # [128, 512]
        out[c * NI : c * NI + 512, :] = y[:F, :].T
        out[c * NI + 512 : (c + 1) * NI, :] = y[F:, :].T
    out = np.ascontiguousarray(out.reshape(N, 2, 32).transpose(1, 0, 2))
    return out, res


def kernel(x, alpha_delta):
    out, _ = run(x, alpha_delta)
    return out


# revision 20
# speedup vs baseline: 1.1086x; 1.0242x over previous
"""Circulant-matmul kernel for Trainium2 (8 NeuronCores, SPMD).

Problem: out[b, i, d] = sum_m alpha[(i - m) mod N] * x[b, m, d]
with x: [2, 8192, 32] fp32, alpha: [8192] fp32.

Strategy (v10, raw bacc, bf16)
------------------------------
Same math as v9: flatten x to X[m, f] (f = b*32 + d, F = 64), shard output
tokens across 8 cores (core c rows [1024c, 1024c+1024)), host-rotate alpha
so every core runs the identical program. 64 accumulating full-array bf16
matmuls compute psum[(h, f), q] = out.T with the pair-stationary trick
([X_j | X_{j+4}] against a 512-wide skewed-alpha moving slice).

v10 changes, driven by the v9 trace (exec 33-38 us):
 * NO TileContext. The Tile scheduler's semaphore plumbing added ~300
   EVENT_SEMAPHORE instructions, ~9 us of which executed AFTER the last
   output DMA and inside the graded exec window (exec_time = last inst end
   minus first engine-op start). Raw bacc with 8 hand-placed semaphores has
   a ~0.5 us tail instead.
 * Paired stationaries [X_j | X_{j+4}] are built by 6 DVE spread-copies
   (BIR requires the matmul stationary AP to have ONE free dim, so the
   direct 3-dim-AP read of xsrc is not allowed), gated per x-chunk with
   one shared semaphore so the PE waits at exactly 3 pair boundaries.
 * wbd is indexed so MM k's moving slice is wbuf[:, 128k : 128k+512] for
   ALL k (the k=63 wrap is materialized at the top of the host view):
   wbd[p, j] = ac[(j + p + 1) mod N], 8576 cols. 3 contiguous wbuf chunks
   + 2 xsrc chunks = 5 input dma_starts (each costs ~0.65 us of serial
   HWDGE descriptor generation, so fewer and bigger is better), issued
   A1,B1,A2,B2,A3 in consumption order across both HWDGE rings.
 * PE waits on the chunk semaphores only at the 4 consumption boundaries
   (k = 0, 13, 24, 37); everything else rides program order.
 * 8 warmup matmuls on a zeroed tile bridge the ~3.4 us HAM un-throttle
   window while the first chunks stream, so real MMs run at 2.4 GHz
   (216 ns per 512-wide bf16 matmul) with no mid-stream re-throttle.
 * psum is drained by two DVE copies (bf16 downcast) feeding one output
   DMA per ring; the program ends on two sync waits for the output DMA
   completion sems.
"""

import os
import sys

import numpy as np

for _p in ("/opt/trn_rl_repo",):
    if os.path.isdir(_p) and _p not in sys.path:
        sys.path.insert(0, _p)

from ml_dtypes import bfloat16

import concourse.bass as bass
from concourse import bacc, bass_utils, mybir

N = 8192          # token axis
P = 128           # SBUF partitions / matmul contraction tile
F = 64            # packed feature dim (B * D = 2 * 32)
NM = N // P       # 64 m-blocks
JW = 8576         # wbuf columns: MM k reads [128k, 128k+512), k = 0..63
NCORES = 8
NI = N // NCORES  # 1024 output rows per core
DT = mybir.dt.bfloat16
# Warmup must bridge from ~7us (DVE memset done) to the first real MM
# (~13us, gated by the B1+spread chain): ~8 cold MMs @427ns until HAM
# un-throttles, then warm @213ns. 18 keeps the PE busy right up to the
# data arrival so the real stream starts warm (the v10b 2.3us PE-idle
# gap re-throttled HAM and cost ~1.5us of cold real MMs).
WARMUP_MM = 15
XSRC_W = (NM + 4) * F  # 4352 (64 blocks + 4 wrap-pad blocks)

# wbuf chunk ends (cols): A1 covers MM k<=12, A2 k<=36, A3 k<=63.
WB_CUTS = (2048, 5120, JW)
# xsrc chunk block boundaries (B1 = [44,68), B2 = [16,44), B3 = [0,16))
# and the pair groups each spread produces. B1 is the first-MM gate:
# smaller B1 -> earlier T0.
XCHUNKS = ((44, 68), (16, 44), (0, 16))
PAIRS = ((44, 64), (16, 44), (0, 16))

_cache = {}


def _build():
    nc = bacc.Bacc(
        "TRN2", target_bir_lowering=False, debug=False, num_devices=NCORES
    )
    xin = nc.dram_tensor("xin", [P, XSRC_W], DT, kind="ExternalInput")
    wbd = nc.dram_tensor("wbd", [P, JW], DT, kind="ExternalInput")
    yout = nc.dram_tensor("yout", [P, 512], DT, kind="ExternalOutput")

    wbuf = nc.alloc_sbuf_tensor("wbuf", [P, JW], DT)
    xsrc = nc.alloc_sbuf_tensor("xsrc", [P, XSRC_W], DT)
    xpair = nc.alloc_sbuf_tensor("xpair", [P, P * NM], DT)
    obuf = nc.alloc_sbuf_tensor("obuf", [P, 512], DT)
    wrm = nc.alloc_sbuf_tensor("wrm", [P, 512], DT)
    ps = nc.alloc_psum_tensor("ps", [P, 512], mybir.dt.float32)
    ps_wrm = nc.alloc_psum_tensor("ps_wrm", [P, 512], mybir.dt.float32)

    s_m = nc.alloc_semaphore("s_m")    # DVE memset -> PE warmup
    s_a = nc.alloc_semaphore("s_a")    # ring A (sync) input chunks
    s_b = nc.alloc_semaphore("s_b")    # ring B (scalar) input chunks
    s_x = nc.alloc_semaphore("s_x")    # spread group done -> PE
    s_pe = nc.alloc_semaphore("s_pe")  # last MM -> DVE drain
    s_c0 = nc.alloc_semaphore("s_c0")  # cast half 0 -> out DMA A
    s_c1 = nc.alloc_semaphore("s_c1")  # cast half 1 -> out DMA B
    s_oa = nc.alloc_semaphore("s_oa")  # out DMA A done
    s_ob = nc.alloc_semaphore("s_ob")  # out DMA B done

    # DVE: zero the warmup stationary/moving tile first thing.
    nc.vector.memset(wrm[:, :], 0.0).then_inc(s_m)

    # Input DMAs, interleaved across the two HWDGE rings in consumption
    # order (each dma_start serializes ~0.65us of descriptor generation on
    # the shared HWDGE block). B1 first: its spread gates the first MM.
    c0, c1, c2 = WB_CUTS
    (b1l, b1h), (b2l, b2h), (b3l, b3h) = XCHUNKS
    nc.scalar.dma_start(
        out=xsrc[:, F * b1l : F * b1h], in_=xin.ap()[:, F * b1l : F * b1h]
    ).then_inc(s_b, 16)
    nc.sync.dma_start(out=wbuf[:, 0:c0], in_=wbd.ap()[:, 0:c0]).then_inc(s_a, 16)
    nc.sync.dma_start(out=wbuf[:, c0:c1], in_=wbd.ap()[:, c0:c1]).then_inc(s_a, 16)
    nc.scalar.dma_start(
        out=xsrc[:, F * b2l : F * b2h], in_=xin.ap()[:, F * b2l : F * b2h]
    ).then_inc(s_b, 16)
    nc.scalar.dma_start(
        out=xsrc[:, F * b3l : F * b3h], in_=xin.ap()[:, F * b3l : F * b3h]
    ).then_inc(s_b, 16)
    nc.sync.dma_start(out=wbuf[:, c1:c2], in_=wbd.ap()[:, c1:c2]).then_inc(s_a, 16)

    # DVE spread-copies build the paired stationaries from each x chunk:
    #   xpair[:, 128j + u]      = xsrc[:, 64j + u]          (u < 64)
    #   xpair[:, 128j + 64 + u] = xsrc[:, 64(j+4) + u]
    xp = xpair[:, :]
    xs = xsrc[:, :]
    for gi, (plo, phi) in enumerate(PAIRS):
        nc.vector.wait_ge(s_b, 16 * (gi + 1))
        nblk = phi - plo
        last = None
        for half, off in ((0, 0), (1, 4 * F)):
            last = nc.vector.tensor_copy(
                bass.AP(
                    xp.tensor,
                    xp.offset + 2 * F * plo + F * half,
                    [[P * NM, P], [2 * F, nblk], [1, F]],
                ),
                bass.AP(
                    xs.tensor,
                    xs.offset + F * plo + off,
                    [[XSRC_W, P], [F, nblk], [1, F]],
                ),
            )
        last.then_inc(s_x)

    # PE warmup: keep the array busy so HAM un-throttles (~3.4us) right as
    # the first input chunks land.
    nc.tensor.wait_ge(s_m, 1)
    for w in range(WARMUP_MM):
        nc.tensor.matmul(
            ps_wrm[:, :],
            lhsT=wrm[:, 0:128],
            rhs=wrm[:, :],
            start=(w == 0),
            stop=(w == WARMUP_MM - 1),
        )

    # Real matmuls: 64 accumulating steps, stationary xpair[:, 128j:128j+128].
    nc.tensor.wait_ge(s_x, 1)
    nc.tensor.wait_ge(s_a, 16)
    mm = None
    for k in range(NM):
        j = NM - 1 - k
        if k == 13:
            nc.tensor.wait_ge(s_a, 32)
        if k == NM - PAIRS[0][0]:   # k=24: pairs j<40 from spread group 2
            nc.tensor.wait_ge(s_x, 2)
        if k == 37:
            nc.tensor.wait_ge(s_a, 48)
        if k == NM - PAIRS[1][0]:   # k=48: pairs j<16 from spread group 3
            nc.tensor.wait_ge(s_x, 3)
        mm = nc.tensor.matmul(
            ps[:, :],
            lhsT=xpair[:, P * j : P * (j + 1)],
            rhs=wbuf[:, P * k : P * k + 512],
            start=(k == 0),
            stop=(k == NM - 1),
        )
    mm.then_inc(s_pe)

    # Drain psum (fp32 -> bf16) on DVE, one half per output DMA ring.
    nc.vector.wait_ge(s_pe, 1)
    nc.vector.tensor_copy(obuf[:, 0:256], ps[:, 0:256]).then_inc(s_c0)
    nc.vector.tensor_copy(obuf[:, 256:512], ps[:, 256:512]).then_inc(s_c1)

    nc.sync.wait_ge(s_c0, 1)
    nc.sync.dma_start(out=yout.ap()[:, 0:256], in_=obuf[:, 0:256]).then_inc(
        s_oa, 16
    )
    nc.scalar.wait_ge(s_c1, 1)
    nc.scalar.dma_start(
        out=yout.ap()[:, 256:512], in_=obuf[:, 256:512]
    ).then_inc(s_ob, 16)

    # Hold the NEFF open until both output DMAs have landed.
    nc.sync.wait_ge(s_oa, 16)
    nc.sync.wait_ge(s_ob, 16)

    nc.compile()
    return nc


def _prep_in_maps(x, alpha_delta):
    X = np.ascontiguousarray(x.transpose(1, 0, 2).reshape(N, F)).astype(bfloat16)
    # Xb[M, p, f] = X[128M + 127 - p, f]   (reversed r-within-block)
    Xb = X.reshape(NM, P, F)[:, ::-1, :]
    Xb = np.concatenate([Xb, Xb[:4]], axis=0)  # wrap pad: X_0..X_3
    xin = np.ascontiguousarray(Xb.transpose(1, 0, 2).reshape(P, XSRC_W))
    in_maps = []
    for c in range(NCORES):
        ac = np.roll(alpha_delta, -NI * c)
        a2 = np.ascontiguousarray(
            np.concatenate([ac, ac, ac[:512]]).astype(bfloat16)
        )
        # host-side skew: wbd[p, j] = a2[N + 1 + p + j], zero-copy view
        wbd = np.ascontiguousarray(
            np.lib.stride_tricks.as_strided(
                a2[N + 1:], shape=(P, JW), strides=(2, 2)
            )
        )
        in_maps.append({"xin": xin, "wbd": wbd})
    return in_maps


def get_nc():
    if "nc" not in _cache:
        _cache["nc"] = _build()
    return _cache["nc"]


def run(x, alpha_delta, **kwargs):
    """Run on hardware; returns (out [2, N, 32], BassKernelResults)."""
    x = np.asarray(x, dtype=np.float32)
    alpha_delta = np.asarray(alpha_delta, dtype=np.float32)
    res = bass_utils.run_bass_kernel_spmd(
        get_nc(), _prep_in_maps(x, alpha_delta), core_ids=list(range(NCORES)),
        **kwargs,
    )
    out = np.empty((N, F), np.float32)
    for c in range(NCORES):
        y = np.asarray(res.results[c]["yout"]).astype(np.float32)  # [128, 512]
        out[c * NI : c * NI + 512, :] = y[:F, :].T
        out[c * NI + 512 : (c + 1) * NI, :] = y[F:, :].T
    out = np.ascontiguousarray(out.reshape(N, 2, 32).transpose(1, 0, 2))
    return out, res


def kernel(x, alpha_delta):
    out, _ = run(x, alpha_delta)
    return out
